# revision 2
# baseline (speedup 1.0000x reference)
"""Trainium2 Bass kernel for nn_MultiHeadAttention (B=4, S=2048, D=768, H=12).

Sharding: query-parallel. 8 cores = 4 batches x 2 query-halves. Each core
computes full K/V projections for its batch plus Q projection / attention /
output projection / LayerNorm for its 1024 query rows. No collectives.

v2 design (vs the ACT-bound v1): the softmax pointwise work is split across
engines so no single engine is saturated:
  head0 of each pair: ACT exp -> fp8e4 p, mask-mult split GP/DVE by columns.
  head1 of each pair: ONE DVE scalar_tensor_tensor: i8 = RNE((log2e*s + 56)
    * keep), bitcast to fp8e4 => p ~= 2^((i8-56)/8) = e^(s/sqrt(dk)).
    (masked -> i8 0 -> +0.0; int8 convert is RNE+saturating, HW-verified)
Both heads' p are fp8 => AV matmuls run fp8 DoubleRow over key-chunk pairs
(half the PE issue time). Rowsum via a 65th ones-column (1/32) in the fp8
V-hat weights; reciprocal gives 32/rs so ctxT is x32-scaled into fp8 range.
Output projection is fp8 DoubleRow (Wo x8 host-side, residual x256); the
resulting 256x psum scale cancels in LayerNorm (scale-invariant; EPS x65536).
All 16 vproj chunks moved to phase 1 (attention has no PE slack anymore).
"""

import sys

for _p in ("/opt/trn_rl_repo", "/root/.axon_site/_ro/trn_rl_repo"):
    if _p not in sys.path:
        sys.path.insert(0, _p)

import numpy as np
import ml_dtypes

B = 4
S = 2048
D = 768
H = 12
DK = 64
NCORES = 8
ROWS = S // 2          # 1024 query rows per core
P = 128
KO = D // P            # 6 contraction chunks
MC = D // P            # 6 head-pair chunks
KC = S // P            # 16 key chunks
RC = ROWS // P         # 8 row chunks
VW = 68                # 64 v cols + ones col + 3 pad (H*VW % 16 == 0 for DR)
EPS = 1e-5
NS = 512               # PSUM bank = 512 f32; matmul out must stay in one bank

LOG2E = 1.4426950408889634
SC_ACT = 1.0 / (8.0 * LOG2E)   # undo log2e scale, apply 1/sqrt(dk)
B_I8 = 56.0                    # fp8e4m3 exponent bias offset (2^0 at i8=56)
ONESC = 1.0 / 32.0             # ones column: rowsum/32 -> recip = 32/rs
QRES_SCALE = 256.0             # 32 (ctx) * 8 (wo) psum scale
EPS_S = EPS * QRES_SCALE * QRES_SCALE
G_GP = 640                     # head0 mask cols on GPSIMD; rest on DVE

BF16 = ml_dtypes.bfloat16

_cached = {}


def _build():
    import concourse.bass as bass
    import concourse.tile as tile
    import concourse.mybir as mybir
    from concourse import bacc

    f32 = mybir.dt.float32
    bf = mybir.dt.bfloat16
    f8 = mybir.dt.float8e4
    i8 = mybir.dt.int8
    AF = mybir.ActivationFunctionType
    OP = mybir.AluOpType
    DR = mybir.MatmulPerfMode.DoubleRow

    nc = bacc.Bacc("TRN2", target_bir_lowering=False, debug=False)

    qt_d = nc.dram_tensor("qt", [D, ROWS], f8, kind="ExternalInput")
    kt_d = nc.dram_tensor("kt", [D, S], f8, kind="ExternalInput")
    vt_d = nc.dram_tensor("vt", [D, S], f8, kind="ExternalInput")
    keep_d = nc.dram_tensor("keep", [S, ROWS], bf, kind="ExternalInput")
    qres_d = nc.dram_tensor("qres", [ROWS, D], bf, kind="ExternalInput")
    w_d = {n: nc.dram_tensor(n, [D, D], f8, kind="ExternalInput")
           for n in ("wq", "wk", "wv", "wo")}
    ident_d = nc.dram_tensor("ident", [P, P], bf, kind="ExternalInput")
    b_d = {n: nc.dram_tensor(n, [D], f32, kind="ExternalInput")
           for n in ("bq", "bk", "gamma", "beta")}
    out_d = nc.dram_tensor("out", [ROWS, D], f32, kind="ExternalOutput")

    rs2_d = [nc.dram_tensor(f"rs2_bounce{mc}", [2, ROWS], bf, kind="Internal")
             for mc in range(MC)]

    def bcast_ap(handle, n, row=0):
        ap = handle.ap()
        return bass.AP(tensor=ap.tensor, offset=row * n, ap=[[0, DK], [1, n]])

    def bcast_ap_p(handle, n):
        ap = handle.ap()
        return bass.AP(tensor=ap.tensor, offset=0, ap=[[0, P], [1, n]])

    with tile.TileContext(nc) as tc:
        with tc.tile_pool(name="wp", bufs=1) as wp, \
             tc.tile_pool(name="xin", bufs=2) as xin, \
             tc.tile_pool(name="kp", bufs=2) as kp, \
             tc.tile_pool(name="ktp", bufs=2) as ktp, \
             tc.tile_pool(name="big", bufs=1) as big, \
             tc.tile_pool(name="pp", bufs=2) as ppool, \
             tc.tile_pool(name="small", bufs=2) as small, \
             tc.tile_pool(name="ph3", bufs=2) as ph3, \
             tc.tile_pool(name="ps", bufs=1, space="PSUM") as psp:

            def s_tile(tag, name):
                return psp.tile([P, ROWS], f32, tag=tag, name=name)

            # ---- PE warm-up in the DMA shadow (~4us to reach HAM 8/8) ----
            warm = wp.tile([P, P], bf, tag="warm")
            nc.vector.memset(warm, 0.0)
            wps = psp.tile([P, NS], f32, tag="sA", name="warm_ps")
            for i in range(36):
                nc.tensor.matmul(wps[:, 0:P], warm, warm,
                                 start=(i == 0), stop=(i == 35))

            # ---- weights / inputs, in PE consumption order ----
            w_sb = {}

            def load_w(n):
                t = wp.tile([P, KO, D], f8, tag=n, name=f"w_{n}")
                src = w_d[n].ap().rearrange("(o p) n -> p o n", p=P)
                for ko in range(KO):
                    nc.sync.dma_start(out=t[:, ko, :], in_=src[:, ko, :])
                w_sb[n] = t

            bq_sb = wp.tile([P, MC], f32, tag="bq")
            bk_sb = wp.tile([P, MC], f32, tag="bk")
            nc.sync.dma_start(out=bq_sb, in_=b_d["bq"].ap().rearrange("(o p) -> p o", p=P))
            nc.sync.dma_start(out=bk_sb, in_=b_d["bk"].ap().rearrange("(o p) -> p o", p=P))
            eps_t = wp.tile([P, 1], f32, tag="eps")
            nc.vector.memset(eps_t, EPS_S)

            # phase 1a: wq + qt first so the PE starts ASAP
            load_w("wq")
            qt_sb = xin.tile([P, KO, ROWS], f8, tag="xin", name="qt_sb")
            qt_src = qt_d.ap().rearrange("(o p) r -> p o r", p=P)
            for ko in range(KO):
                nc.sync.dma_start(out=qt_sb[:, ko, :], in_=qt_src[:, ko, :])

            load_w("wk")
            kt_sb = []
            kt_src = kt_d.ap().rearrange("(o p) s -> p o s", p=P)
            for half in range(2):
                t = ktp.tile([P, KO, ROWS], f8, tag="kt", name=f"kt_sb{half}")
                for ko in range(KO):
                    nc.sync.dma_start(
                        out=t[:, ko, :],
                        in_=kt_src[:, ko, half * ROWS:(half + 1) * ROWS])
                kt_sb.append(t)

            load_w("wv")
            vt_sb = []
            vt_src = vt_d.ap().rearrange("(o p) s -> p o s", p=P)
            for half in range(2):
                t = xin.tile([P, KO, ROWS], f8, tag="xin", name=f"vt_sb{half}")
                for ko in range(KO):
                    nc.sync.dma_start(
                        out=t[:, ko, :],
                        in_=vt_src[:, ko, half * ROWS:(half + 1) * ROWS])
                vt_sb.append(t)

            # keep mask rides the scalar engine's DMA queue; chunks 0-3 early
            keep_sb = big.tile([P, KC, ROWS], bf, tag="keep")
            keep_src = keep_d.ap().rearrange("(c p) r -> p c r", p=P)
            for c in range(4):
                nc.scalar.dma_start(out=keep_sb[:, c, :], in_=keep_src[:, c, :])

            bvb_gb = wp.tile([P, D], bf, tag="gb")
            bb = wp.tile([P, D], bf, tag="bb")
            nc.gpsimd.dma_start(out=bvb_gb, in_=bcast_ap_p(b_d["gamma"], D))
            nc.gpsimd.dma_start(out=bb, in_=bcast_ap_p(b_d["beta"], D))
            gb = bvb_gb

            for c in range(4, KC):
                nc.scalar.dma_start(out=keep_sb[:, c, :], in_=keep_src[:, c, :])
            load_w("wo")
            ident_sb = wp.tile([P, P], bf, tag="ident")
            nc.gpsimd.dma_start(out=ident_sb, in_=ident_d.ap())
            qres_sb = big.tile([P, RC, D], bf, tag="qres")
            qres_src = qres_d.ap().rearrange("(c p) d -> p c d", p=P)
            for rc in range(RC):
                nc.gpsimd.dma_start(out=qres_sb[:, rc, :], in_=qres_src[:, rc, :])

            # ---- phase 1: qhat = log2e * (q + bq), bf16 ----
            qhat = big.tile([P, MC, ROWS], bf, tag="qhat")
            for mc in range(MC):
                ps = s_tile("sA" if mc % 2 == 0 else "sB", f"qh_{mc}")
                for kp2 in range(KO // 2):
                    for n0 in (0, NS):
                        nc.tensor.matmul(
                            ps[:, n0:n0 + NS],
                            w_sb["wq"][:, 2 * kp2:2 * kp2 + 2, mc * P:(mc + 1) * P],
                            qt_sb[:, 2 * kp2:2 * kp2 + 2, n0:n0 + NS],
                            start=(kp2 == 0), stop=(kp2 == KO // 2 - 1),
                            perf_mode=DR)
                nc.scalar.activation(out=qhat[:, mc, :], in_=ps, func=AF.Identity,
                                     bias=bq_sb[:, mc:mc + 1], scale=LOG2E)

            # ---- kproj: kbuf[mc] = k~^T + bk (bf16), copies on ACT ----
            kbufs = [None] * MC

            def kproj_group(mc, g, tag="cx"):
                half, q0 = divmod(g, 2)
                ps = psp.tile([P, NS], f32, tag=tag, bufs=4 if tag == "cx" else 1,
                              name=f"kp_{mc}_{g}")
                for kp2 in range(KO // 2):
                    nc.tensor.matmul(
                        ps,
                        w_sb["wk"][:, 2 * kp2:2 * kp2 + 2, mc * P:(mc + 1) * P],
                        kt_sb[half][:, 2 * kp2:2 * kp2 + 2, q0 * NS:(q0 + 1) * NS],
                        start=(kp2 == 0), stop=(kp2 == KO // 2 - 1),
                        perf_mode=DR)
                nc.scalar.activation(out=kbufs[mc][:, g * NS:(g + 1) * NS],
                                     in_=ps, func=AF.Identity,
                                     bias=bk_sb[:, mc:mc + 1], scale=1.0)

            kbufs[0] = kp.tile([P, S], bf, tag="kbuf", name="kbuf_0")
            for g in range(4):
                kproj_group(0, g)

            # ---- v~ fp8 with ones column (all 16 chunks in phase 1) ----
            vhat = big.tile([P, KC, H * VW], f8, tag="vhat")
            vh4 = vhat.rearrange("p c (h w) -> p c h w", w=VW)
            with nc.allow_low_precision(reason="fp8 vhat"):
                nc.vector.memset(vh4[:, :, :, DK:DK + 1], ONESC)

            def vproj(kc, tag):
                half, c = divmod(kc, KC // 2)
                ps = psp.tile([P, D], f32, tag=tag, name=f"vp_{kc}")
                for kp2 in range(KO // 2):
                    for n0 in (0, NS):
                        n1 = min(n0 + NS, D)
                        nc.tensor.matmul(
                            ps[:, n0:n1],
                            vt_sb[half][:, 2 * kp2:2 * kp2 + 2, c * P:(c + 1) * P],
                            w_sb["wv"][:, 2 * kp2:2 * kp2 + 2, n0:n1],
                            start=(kp2 == 0), stop=(kp2 == KO // 2 - 1),
                            perf_mode=DR)
                with nc.allow_low_precision(reason="fp8 vhat"):
                    nc.scalar.activation(
                        out=vh4[:, kc, :, 0:DK],
                        in_=ps.rearrange("p (h w) -> p h w", w=DK),
                        func=AF.Copy)

            for kc in range(KC):
                vproj(kc, "sA" if kc % 2 == 0 else "sB")

            # ---- attention: head pairs ----------------------------------
            ctxT = big.tile([P, MC, ROWS], f8, tag="ctxT")
            rsb_tiles = {}
            rsT_tiles = {}

            def recip_pair(mc):
                rsT = rsT_tiles.pop(mc)
                with nc.allow_low_precision(reason="softmax rowsum recip bf16"):
                    nc.vector.reciprocal(out=rsT, in_=rsT)
                nc.sync.dma_start(
                    out=rs2_d[mc].ap().rearrange("a b -> (a b)")
                    .rearrange("(p o) -> p o", p=P), in_=rsT)
                rsb = small.tile([P, ROWS], bf, tag="rsb", name=f"rsb_{mc}")
                rsb_tiles[mc] = rsb
                for hh in range(2):
                    nc.gpsimd.dma_start(
                        out=rsb[hh * DK:(hh + 1) * DK, :],
                        in_=bcast_ap(rs2_d[mc], ROWS, row=hh))

            def finish_pair(mc):
                # normalize ctxT for pair mc on GPSIMD (x 32/rowsum)
                with nc.allow_low_precision(reason="fp8 ctxT"):
                    nc.gpsimd.tensor_tensor(out=ctxT[:, mc, :], in0=ctxT[:, mc, :],
                                            in1=rsb_tiles.pop(mc), op=OP.mult)

            for mc in range(MC):
                kbuf = kbufs[mc]
                ctx = [[psp.tile([DK + 1, NS], f32, tag="cx", bufs=4,
                                 name=f"ctx_{mc}_{hh}_{qh}")
                        for qh in range(2)] for hh in range(2)]
                p_pair = [None, None]  # fp8 [P, 2, ROWS] per head, per kc-pair
                prev_pair = None

                def do_av(j, pair_tiles):
                    for hh in range(2):
                        h = 2 * mc + hh
                        for qh in range(2):
                            nc.tensor.matmul(
                                ctx[hh][qh],
                                vh4[:, 2 * j:2 * j + 2, h, 0:DK + 1],
                                pair_tiles[hh][:, :, qh * NS:(qh + 1) * NS],
                                start=(j == 0), stop=(j == KC // 2 - 1),
                                perf_mode=DR)

                for kc in range(KC):
                    # deferred rowsum recip / normalize, off critical path
                    if kc == 5 and mc > 0:
                        recip_pair(mc - 1)
                    if kc == 11 and mc > 0:
                        finish_pair(mc - 1)

                    # concurrent row-tiled scores for the two heads
                    s_ps = [s_tile("sA", f"s_{mc}_{kc}_0"),
                            s_tile("sB", f"s_{mc}_{kc}_1")]
                    for n0 in (0, NS):
                        for hh in range(2):
                            pr = slice(hh * DK, (hh + 1) * DK)
                            nc.tensor.matmul(s_ps[hh][:, n0:n0 + NS],
                                             kbuf[pr, kc * P:(kc + 1) * P],
                                             qhat[pr, mc, n0:n0 + NS],
                                             start=True, stop=True)
                    # AV for the previous completed kc-pair (lags pointwise)
                    if kc % 2 == 0:
                        if kc >= 2:
                            do_av(kc // 2 - 1, prev_pair)
                        prev_pair = [
                            ppool.tile([P, 2, ROWS], f8, tag=f"p{hh}",
                                       name=f"p_{mc}_{kc // 2}_{hh}")
                            for hh in range(2)]
                    jj = kc & 1
                    p0, p1 = prev_pair

                    # head0: ACT exp -> fp8, mask split GP / DVE
                    with nc.allow_low_precision(reason="fp8 softmax"):
                        nc.scalar.activation(out=p0[:, jj, :], in_=s_ps[0],
                                             func=AF.Exp, scale=SC_ACT)
                        nc.gpsimd.tensor_tensor(
                            out=p0[:, jj, 0:G_GP], in0=p0[:, jj, 0:G_GP],
                            in1=keep_sb[:, kc, 0:G_GP], op=OP.mult)
                        nc.vector.tensor_tensor(
                            out=p0[:, jj, G_GP:ROWS], in0=p0[:, jj, G_GP:ROWS],
                            in1=keep_sb[:, kc, G_GP:ROWS], op=OP.mult)
                        # head1: one-op exp2 bit-trick with fused mask
                        nc.vector.scalar_tensor_tensor(
                            out=p1[:, jj, :].bitcast(i8), in0=s_ps[1],
                            scalar=B_I8, in1=keep_sb[:, kc, :],
                            op0=OP.add, op1=OP.mult)
                do_av(KC // 2 - 1, prev_pair)

                # pair boundary: kproj of the next pair rides freed slots
                if mc + 1 < MC:
                    kbufs[mc + 1] = kp.tile([P, S], bf, tag="kbuf",
                                            name=f"kbuf_{mc + 1}")
                    kproj_group(mc + 1, 0, tag="sB")  # sB frees first
                rs_t = small.tile([DK + 1, 2 * ROWS], bf, tag="rs", bufs=1,
                                  name=f"rs_{mc}")
                for hh in range(2):
                    for qh in range(2):
                        nc.scalar.activation(
                            out=rs_t[DK:DK + 1,
                                     hh * ROWS + qh * NS:hh * ROWS + (qh + 1) * NS],
                            in_=ctx[hh][qh][DK:DK + 1, :], func=AF.Copy)
                rsT = small.tile([P, 2 * ROWS // P], bf, tag="rsT",
                                 name=f"rsT_{mc}")
                rsT_tiles[mc] = rsT
                nc.sync.dma_start(out=rsT, in_=rs_t[DK:DK + 1, :])
                for g, (hh, qh) in enumerate(((0, 0), (0, 1), (1, 0), (1, 1))):
                    pr = slice(hh * DK, (hh + 1) * DK)
                    with nc.allow_low_precision(reason="fp8 ctxT"):
                        nc.scalar.activation(
                            out=ctxT[pr, mc, qh * NS:(qh + 1) * NS],
                            in_=ctx[hh][qh][0:DK, :], func=AF.Copy)
                    if mc + 1 < MC and g >= 1:
                        kproj_group(mc + 1, g)
            # preload sqrt table set while the last bounce is in flight
            sq_warm = small.tile([1, 1], f32, tag="sqw")
            nc.scalar.activation(out=sq_warm, in_=eps_t[0:1, 0:1], func=AF.Sqrt,
                                 bias=eps_t[0:1, 0:1], scale=1.0)
            recip_pair(MC - 1)

            # ---- phase 3: out projection (fp8 DR) + residual + LayerNorm
            # DR pairs (0,1),(2,3),(4,5): the last pair's ctxT chunk (5, whose
            # normalization lands latest) stays in the final group; rc0/rc1's
            # first two groups run under the last normalize chain.
            op_ps = {}

            def oproj_part(rc, gs, last=False):
                if rc not in op_ps:
                    op_ps[rc] = psp.tile([P, D], f32,
                                         tag="sA" if rc % 2 == 0 else "sB",
                                         name=f"op_{rc}")
                for g in gs:
                    for n0 in (0, NS):
                        n1 = min(n0 + NS, D)
                        nc.tensor.matmul(
                            op_ps[rc][:, n0:n1],
                            ctxT[:, 2 * g:2 * g + 2, rc * P:(rc + 1) * P],
                            w_sb["wo"][:, 2 * g:2 * g + 2, n0:n1],
                            start=(g == 0), stop=False, perf_mode=DR)
                if last:
                    # residual add (x256 host-side) via identity matmul
                    for n0 in (0, NS):
                        n1 = min(n0 + NS, D)
                        nc.tensor.matmul(op_ps[rc][:, n0:n1], ident_sb,
                                         qres_sb[:, rc, n0:n1],
                                         start=False, stop=True)

            oproj_part(0, range(2))
            oproj_part(1, range(2))
            finish_pair(MC - 1)
            nsub = 2
            sub = D // nsub  # 384 <= BN_STATS_FMAX
            for rc in range(RC):
                if rc < 2:
                    oproj_part(rc, [2], last=True)
                else:
                    oproj_part(rc, range(3), last=True)
                ps = op_ps.pop(rc)

                x_t = ph3.tile([P, D], bf, tag="x")
                nc.scalar.activation(out=x_t, in_=ps, func=AF.Copy)
                stats = small.tile([P, nsub, 6], f32, tag="stats")
                for sg in range(nsub):
                    nc.vector.bn_stats(out=stats[:, sg, :],
                                       in_=x_t[:, sg * sub:(sg + 1) * sub])
                mv = small.tile([P, 2], f32, tag="mv")
                nc.vector.bn_aggr(out=mv, in_=stats)
                std_t = small.tile([P, 1], f32, tag="std")
                nc.scalar.activation(out=std_t, in_=mv[:, 1:2], func=AF.Sqrt,
                                     bias=eps_t, scale=1.0)
                nc.vector.reciprocal(out=std_t, in_=std_t)
                xn = ph3.tile([P, D], bf, tag="xn")
                nc.vector.tensor_scalar(out=xn, in0=x_t, scalar1=mv[:, 0:1],
                                        scalar2=std_t, op0=OP.subtract,
                                        op1=OP.mult)
                xg = ph3.tile([P, D], bf, tag="xg")
                nc.gpsimd.tensor_tensor(out=xg, in0=xn, in1=gb, op=OP.mult)
                xf = ph3.tile([P, D], f32, tag="xf")
                nc.vector.tensor_tensor(out=xf, in0=xg, in1=bb, op=OP.add)
                nc.sync.dma_start(out=out_d.ap()[rc * P:(rc + 1) * P, :], in_=xf)

    nc.compile()
    return nc


def _get_nc():
    if "nc" not in _cached:
        _cached["nc"] = _build()
    return _cached["nc"]


def _make_in_maps(Q, Kt, V, attn_mask, Wq, bq, Wk, bk, Wv, bv, Wo, bo, gamma, beta):
    f32 = np.float32
    FP8 = ml_dtypes.float8_e4m3
    # fp8 DoubleRow projections: weights x8 / activations /8 so products
    # stay in range; wo x8 with the ctx x32 scale cancelled by LayerNorm.
    w = {"wq": (np.ascontiguousarray(Wq, f32) * 8).astype(FP8),
         "wk": (np.ascontiguousarray(Wk, f32) * 8).astype(FP8),
         "wv": (np.ascontiguousarray(Wv, f32) * 8).astype(FP8),
         "wo": (np.ascontiguousarray(Wo, f32) * 8).astype(FP8)}
    b = {"bq": np.ascontiguousarray(bq, f32) * np.float32(LOG2E),
         "bk": np.ascontiguousarray(bk, f32),
         "gamma": np.ascontiguousarray(gamma, f32),
         "beta": np.ascontiguousarray(beta, f32)}
    # fold bv and bo into the residual: out = attnV@Wo + (Q + bo + bv@Wo)
    res_bias = (np.asarray(bo, f32) + np.asarray(bv, f32) @ np.asarray(Wo, f32))
    in_maps = []
    for c in range(NCORES):
        bidx, half = divmod(c, 2)
        rows = slice(half * ROWS, (half + 1) * ROWS)
        m = {
            "qt": (np.ascontiguousarray(Q[bidx, rows].T) / 8).astype(FP8),
            "kt": (np.ascontiguousarray(Kt[bidx].T) / 8).astype(FP8),
            "vt": (np.ascontiguousarray(V[bidx].T) / 8).astype(FP8),
            "keep": np.ascontiguousarray(
                (~attn_mask[bidx, rows]).T.astype(BF16)),
            "qres": ((np.ascontiguousarray(Q[bidx, rows], f32) + res_bias)
                     * np.float32(QRES_SCALE)).astype(BF16),
        }
        m["ident"] = np.eye(P, dtype=BF16)
        m.update(w)
        m.update(b)
        in_maps.append(m)
    return in_maps


def kernel(Q, K, V, attn_mask, Wq, bq, Wk, bk, Wv, bv, Wo, bo, gamma, beta,
           _profile=None):
    from concourse.bass_utils import run_bass_kernel_spmd

    nc = _get_nc()
    in_maps = _make_in_maps(np.asarray(Q, np.float32), np.asarray(K, np.float32),
                            np.asarray(V, np.float32), np.asarray(attn_mask),
                            Wq, bq, Wk, bk, Wv, bv, Wo, bo, gamma, beta)
    kwargs = dict(_profile) if _profile else {}
    res = run_bass_kernel_spmd(nc, in_maps, list(range(NCORES)), **kwargs)
    if _profile is not None:
        _cached["last_results"] = res
    out = np.empty((B, S, D), np.float32)
    for c, m in enumerate(res.results):
        bidx, half = divmod(c, 2)
        out[bidx, half * ROWS:(half + 1) * ROWS] = m["out"]
    return out


# revision 5
# speedup vs baseline: 1.2290x; 1.2290x over previous
"""Trainium2 Bass kernel for nn_MultiHeadAttention (B=4, S=2048, D=768, H=12).

Sharding: query-parallel. 8 cores = 4 batches x 2 query-halves. Each core
computes full K/V projections for its batch plus Q projection / attention /
output projection / LayerNorm for its 1024 query rows. No collectives.

v2 design (vs the ACT-bound v1): the softmax pointwise work is split across
engines so no single engine is saturated:
  head0 of each pair: ACT exp -> fp8e4 p, mask-mult split GP/DVE by columns.
  head1 of each pair: ONE DVE scalar_tensor_tensor: i8 = RNE((log2e*s + 56)
    * keep), bitcast to fp8e4 => p ~= 2^((i8-56)/8) = e^(s/sqrt(dk)).
    (masked -> i8 0 -> +0.0; int8 convert is RNE+saturating, HW-verified)
Both heads' p are fp8 => AV matmuls run fp8 DoubleRow over key-chunk pairs
(half the PE issue time). Rowsum via a 65th ones-column (1/32) in the fp8
V-hat weights; reciprocal gives 32/rs so ctxT is x32-scaled into fp8 range.
Output projection is fp8 DoubleRow (Wo x8 host-side, residual x256); the
resulting 256x psum scale cancels in LayerNorm (scale-invariant; EPS x65536).
All 16 vproj chunks moved to phase 1 (attention has no PE slack anymore).
"""

import sys

for _p in ("/opt/trn_rl_repo", "/root/.axon_site/_ro/trn_rl_repo"):
    if _p not in sys.path:
        sys.path.insert(0, _p)

import numpy as np
import ml_dtypes

B = 4
S = 2048
D = 768
H = 12
DK = 64
NCORES = 8
ROWS = S // 2          # 1024 query rows per core
P = 128
KO = D // P            # 6 contraction chunks
MC = D // P            # 6 head-pair chunks
KC = S // P            # 16 key chunks
RC = ROWS // P         # 8 row chunks
VW = 68                # 64 v cols + ones col + 3 pad (H*VW % 16 == 0 for DR)
EPS = 1e-5
NS = 512               # PSUM bank = 512 f32; matmul out must stay in one bank

LOG2E = 1.4426950408889634
SC_ACT = 1.0 / (8.0 * LOG2E)   # undo log2e scale, apply 1/sqrt(dk)
B_I8 = 56.0                    # fp8e4m3 exponent bias offset (2^0 at i8=56)
ONESC = 1.0 / 32.0             # ones column: rowsum/32 -> recip = 32/rs
QRES_SCALE = 256.0             # 32 (ctx) * 8 (wo) psum scale
EPS_S = EPS * QRES_SCALE * QRES_SCALE
G_GP = 320                     # head0 mask cols on GPSIMD; rest on DVE

BF16 = ml_dtypes.bfloat16

_cached = {}


def _build():
    import concourse.bass as bass
    import concourse.tile as tile
    import concourse.mybir as mybir
    from concourse import bacc

    f32 = mybir.dt.float32
    bf = mybir.dt.bfloat16
    f8 = mybir.dt.float8e4
    i8 = mybir.dt.int8
    AF = mybir.ActivationFunctionType
    OP = mybir.AluOpType
    DR = mybir.MatmulPerfMode.DoubleRow

    nc = bacc.Bacc("TRN2", target_bir_lowering=False, debug=False)

    qt_d = nc.dram_tensor("qt", [D, ROWS], f8, kind="ExternalInput")
    kt_d = nc.dram_tensor("kt", [D, S], f8, kind="ExternalInput")
    vt_d = nc.dram_tensor("vt", [D, S], f8, kind="ExternalInput")
    keep_d = nc.dram_tensor("keep", [S, ROWS], bf, kind="ExternalInput")
    qres_d = nc.dram_tensor("qres", [ROWS, D], bf, kind="ExternalInput")
    w_d = {n: nc.dram_tensor(n, [D, D], f8, kind="ExternalInput")
           for n in ("wq", "wk", "wv", "wo")}
    ident_d = nc.dram_tensor("ident", [P, P], bf, kind="ExternalInput")
    b_d = {n: nc.dram_tensor(n, [D], f32, kind="ExternalInput")
           for n in ("bq", "bk", "gamma", "beta")}
    out_d = nc.dram_tensor("out", [ROWS, D], f32, kind="ExternalOutput")

    rs2_d = [nc.dram_tensor(f"rs2_bounce{mc}", [2, ROWS], bf, kind="Internal")
             for mc in range(MC)]

    def bcast_ap(handle, n, row=0):
        ap = handle.ap()
        return bass.AP(tensor=ap.tensor, offset=row * n, ap=[[0, DK], [1, n]])

    def bcast_ap_p(handle, n):
        ap = handle.ap()
        return bass.AP(tensor=ap.tensor, offset=0, ap=[[0, P], [1, n]])

    with tile.TileContext(nc) as tc:
        with tc.tile_pool(name="wp", bufs=1) as wp, \
             tc.tile_pool(name="xin", bufs=2) as xin, \
             tc.tile_pool(name="kp", bufs=2) as kp, \
             tc.tile_pool(name="ktp", bufs=2) as ktp, \
             tc.tile_pool(name="big", bufs=1) as big, \
             tc.tile_pool(name="pp", bufs=2) as ppool, \
             tc.tile_pool(name="small", bufs=2) as small, \
             tc.tile_pool(name="ph3", bufs=2) as ph3, \
             tc.tile_pool(name="ps", bufs=1, space="PSUM") as psp:

            def s_tile(tag, name):
                return psp.tile([P, ROWS], f32, tag=tag, name=name)

            # ---- PE warm-up in the DMA shadow (~4us to reach HAM 8/8) ----
            warm = wp.tile([P, P], bf, tag="warm")
            nc.vector.memset(warm, 0.0)
            wps = psp.tile([P, NS], f32, tag="sA", name="warm_ps")
            for i in range(36):
                nc.tensor.matmul(wps[:, 0:P], warm, warm,
                                 start=(i == 0), stop=(i == 35))

            # ---- weights / inputs, in PE consumption order ----
            w_sb = {}

            def load_w(n):
                t = wp.tile([P, KO, D], f8, tag=n, name=f"w_{n}")
                src = w_d[n].ap().rearrange("(o p) n -> p o n", p=P)
                for ko in range(KO):
                    nc.sync.dma_start(out=t[:, ko, :], in_=src[:, ko, :])
                w_sb[n] = t

            bq_sb = wp.tile([P, MC], f32, tag="bq")
            bk_sb = wp.tile([P, MC], f32, tag="bk")
            nc.sync.dma_start(out=bq_sb, in_=b_d["bq"].ap().rearrange("(o p) -> p o", p=P))
            nc.sync.dma_start(out=bk_sb, in_=b_d["bk"].ap().rearrange("(o p) -> p o", p=P))
            eps_t = wp.tile([P, 1], f32, tag="eps")
            nc.vector.memset(eps_t, EPS_S)

            # phase 1a: wq + qt first so the PE starts ASAP
            load_w("wq")
            qt_sb = xin.tile([P, KO, ROWS], f8, tag="xin", name="qt_sb")
            qt_src = qt_d.ap().rearrange("(o p) r -> p o r", p=P)
            for ko in range(KO):
                nc.sync.dma_start(out=qt_sb[:, ko, :], in_=qt_src[:, ko, :])

            load_w("wk")
            kt_sb = []
            kt_src = kt_d.ap().rearrange("(o p) s -> p o s", p=P)
            for half in range(2):
                t = ktp.tile([P, KO, ROWS], f8, tag="kt", name=f"kt_sb{half}")
                for ko in range(KO):
                    nc.sync.dma_start(
                        out=t[:, ko, :],
                        in_=kt_src[:, ko, half * ROWS:(half + 1) * ROWS])
                kt_sb.append(t)

            load_w("wv")
            vt_sb = []
            vt_src = vt_d.ap().rearrange("(o p) s -> p o s", p=P)
            for half in range(2):
                t = xin.tile([P, KO, ROWS], f8, tag="xin", name=f"vt_sb{half}")
                for ko in range(KO):
                    nc.sync.dma_start(
                        out=t[:, ko, :],
                        in_=vt_src[:, ko, half * ROWS:(half + 1) * ROWS])
                vt_sb.append(t)

            # keep mask rides the scalar engine's DMA queue; chunks 0-3 early
            keep_sb = big.tile([P, KC, ROWS], bf, tag="keep")
            keep_src = keep_d.ap().rearrange("(c p) r -> p c r", p=P)
            for c in range(4):
                nc.scalar.dma_start(out=keep_sb[:, c, :], in_=keep_src[:, c, :])

            bvb_gb = wp.tile([P, D], bf, tag="gb")
            bb = wp.tile([P, D], bf, tag="bb")
            nc.gpsimd.dma_start(out=bvb_gb, in_=bcast_ap_p(b_d["gamma"], D))
            nc.gpsimd.dma_start(out=bb, in_=bcast_ap_p(b_d["beta"], D))
            gb = bvb_gb

            for c in range(4, KC):
                nc.scalar.dma_start(out=keep_sb[:, c, :], in_=keep_src[:, c, :])
            load_w("wo")
            ident_sb = wp.tile([P, P], bf, tag="ident")
            nc.gpsimd.dma_start(out=ident_sb, in_=ident_d.ap())
            qres_sb = big.tile([P, RC, D], bf, tag="qres")
            qres_src = qres_d.ap().rearrange("(c p) d -> p c d", p=P)
            for rc in range(RC):
                nc.gpsimd.dma_start(out=qres_sb[:, rc, :], in_=qres_src[:, rc, :])

            # ---- phase 1: qhat = log2e * (q + bq), bf16 ----
            qhat = big.tile([P, MC, ROWS], bf, tag="qhat")
            for mc in range(MC):
                ps = s_tile("sA" if mc % 2 == 0 else "sB", f"qh_{mc}")
                for kp2 in range(KO // 2):
                    for n0 in (0, NS):
                        nc.tensor.matmul(
                            ps[:, n0:n0 + NS],
                            w_sb["wq"][:, 2 * kp2:2 * kp2 + 2, mc * P:(mc + 1) * P],
                            qt_sb[:, 2 * kp2:2 * kp2 + 2, n0:n0 + NS],
                            start=(kp2 == 0), stop=(kp2 == KO // 2 - 1),
                            perf_mode=DR)
                nc.scalar.activation(out=qhat[:, mc, :], in_=ps, func=AF.Identity,
                                     bias=bq_sb[:, mc:mc + 1], scale=LOG2E)

            # ---- kproj: kbuf[mc] = k~^T + bk (bf16), copies on ACT ----
            kbufs = [None] * MC

            def kproj_group(mc, g, tag="cx"):
                half, q0 = divmod(g, 2)
                ps = psp.tile([P, NS], f32, tag=tag, bufs=4 if tag == "cx" else 1,
                              name=f"kp_{mc}_{g}")
                for kp2 in range(KO // 2):
                    nc.tensor.matmul(
                        ps,
                        w_sb["wk"][:, 2 * kp2:2 * kp2 + 2, mc * P:(mc + 1) * P],
                        kt_sb[half][:, 2 * kp2:2 * kp2 + 2, q0 * NS:(q0 + 1) * NS],
                        start=(kp2 == 0), stop=(kp2 == KO // 2 - 1),
                        perf_mode=DR)
                nc.scalar.activation(out=kbufs[mc][:, g * NS:(g + 1) * NS],
                                     in_=ps, func=AF.Identity,
                                     bias=bk_sb[:, mc:mc + 1], scale=1.0)

            kbufs[0] = kp.tile([P, S], bf, tag="kbuf", name="kbuf_0")
            for g in range(4):
                kproj_group(0, g)

            # ---- v~ fp8 with ones column (all 16 chunks in phase 1) ----
            vhat = big.tile([P, KC, H * VW], f8, tag="vhat")
            vh4 = vhat.rearrange("p c (h w) -> p c h w", w=VW)
            with nc.allow_low_precision(reason="fp8 vhat"):
                nc.vector.memset(vh4[:, :, :, DK:DK + 1], ONESC)

            def vproj(kc, tag):
                half, c = divmod(kc, KC // 2)
                ps = psp.tile([P, D], f32, tag=tag, name=f"vp_{kc}")
                for kp2 in range(KO // 2):
                    for n0 in (0, NS):
                        n1 = min(n0 + NS, D)
                        nc.tensor.matmul(
                            ps[:, n0:n1],
                            vt_sb[half][:, 2 * kp2:2 * kp2 + 2, c * P:(c + 1) * P],
                            w_sb["wv"][:, 2 * kp2:2 * kp2 + 2, n0:n1],
                            start=(kp2 == 0), stop=(kp2 == KO // 2 - 1),
                            perf_mode=DR)
                with nc.allow_low_precision(reason="fp8 vhat"):
                    nc.scalar.activation(
                        out=vh4[:, kc, :, 0:DK],
                        in_=ps.rearrange("p (h w) -> p h w", w=DK),
                        func=AF.Copy)

            for kc in range(KC):
                vproj(kc, "sA" if kc % 2 == 0 else "sB")

            # ---- attention: head pairs ----------------------------------
            ctxT = big.tile([P, MC, ROWS], f8, tag="ctxT")
            rsb_tiles = {}
            rsT_tiles = {}

            def recip_pair(mc):
                rsT = rsT_tiles.pop(mc)
                with nc.allow_low_precision(reason="softmax rowsum recip bf16"):
                    nc.vector.reciprocal(out=rsT, in_=rsT)
                nc.sync.dma_start(
                    out=rs2_d[mc].ap().rearrange("a b -> (a b)")
                    .rearrange("(p o) -> p o", p=P), in_=rsT)
                rsb = small.tile([P, ROWS], bf, tag="rsb", name=f"rsb_{mc}")
                rsb_tiles[mc] = rsb
                for hh in range(2):
                    nc.gpsimd.dma_start(
                        out=rsb[hh * DK:(hh + 1) * DK, :],
                        in_=bcast_ap(rs2_d[mc], ROWS, row=hh))

            def finish_pair(mc):
                # normalize ctxT for pair mc on GPSIMD (x 32/rowsum)
                with nc.allow_low_precision(reason="fp8 ctxT"):
                    nc.gpsimd.tensor_tensor(out=ctxT[:, mc, :], in0=ctxT[:, mc, :],
                                            in1=rsb_tiles.pop(mc), op=OP.mult)

            for mc in range(MC):
                kbuf = kbufs[mc]
                ctx = [[psp.tile([DK + 1, NS], f32, tag="cx", bufs=4,
                                 name=f"ctx_{mc}_{hh}_{qh}")
                        for qh in range(2)] for hh in range(2)]
                p_pair = [None, None]  # fp8 [P, 2, ROWS] per head, per kc-pair
                prev_pair = None

                def do_av(j, pair_tiles):
                    # head0: bf16 p, plain MMs; head1: fp8 p, DoubleRow
                    h0 = 2 * mc
                    for jj2 in range(2):
                        for qh in range(2):
                            nc.tensor.matmul(
                                ctx[0][qh],
                                vh4[:, 2 * j + jj2, h0, 0:DK + 1],
                                pair_tiles[0][:, jj2, qh * NS:(qh + 1) * NS],
                                start=(j == 0 and jj2 == 0),
                                stop=(j == KC // 2 - 1 and jj2 == 1))
                    for qh in range(2):
                        nc.tensor.matmul(
                            ctx[1][qh],
                            vh4[:, 2 * j:2 * j + 2, 2 * mc + 1, 0:DK + 1],
                            pair_tiles[1][:, :, qh * NS:(qh + 1) * NS],
                            start=(j == 0), stop=(j == KC // 2 - 1),
                            perf_mode=DR)

                for kc in range(KC):
                    # deferred rowsum recip / normalize, off critical path
                    if kc == 5 and mc > 0:
                        recip_pair(mc - 1)
                    if kc == 11 and mc > 0:
                        finish_pair(mc - 1)

                    # concurrent row-tiled scores for the two heads
                    s_ps = [s_tile("sA", f"s_{mc}_{kc}_0"),
                            s_tile("sB", f"s_{mc}_{kc}_1")]
                    for n0 in (0, NS):
                        for hh in range(2):
                            pr = slice(hh * DK, (hh + 1) * DK)
                            nc.tensor.matmul(s_ps[hh][:, n0:n0 + NS],
                                             kbuf[pr, kc * P:(kc + 1) * P],
                                             qhat[pr, mc, n0:n0 + NS],
                                             start=True, stop=True)
                    # AV for the previous completed kc-pair (lags pointwise)
                    if kc % 2 == 0:
                        if kc >= 2:
                            do_av(kc // 2 - 1, prev_pair)
                        prev_pair = [
                            ppool.tile([P, 2, ROWS], bf, tag="p0",
                                       name=f"p_{mc}_{kc // 2}_0"),
                            ppool.tile([P, 2, ROWS], f8, tag="p1",
                                       name=f"p_{mc}_{kc // 2}_1")]
                    jj = kc & 1
                    p0, p1 = prev_pair

                    # head0: ACT exp -> bf16 (mask applied per kc-pair below)
                    nc.scalar.activation(out=p0[:, jj, :], in_=s_ps[0],
                                         func=AF.Exp, scale=SC_ACT)
                    # head1: one-op exp2 bit-trick with fused mask
                    with nc.allow_low_precision(reason="fp8 softmax"):
                        nc.vector.scalar_tensor_tensor(
                            out=p1[:, jj, :].bitcast(i8), in0=s_ps[1],
                            scalar=B_I8, in1=keep_sb[:, kc, :],
                            op0=OP.add, op1=OP.mult)
                    if jj == 1:
                        # head0 mask, batched over the kc-pair: GP low cols,
                        # DVE high cols (bf16 2x)
                        kcp = slice(kc - 1, kc + 1)
                        nc.gpsimd.tensor_tensor(
                            out=p0[:, :, 0:G_GP], in0=p0[:, :, 0:G_GP],
                            in1=keep_sb[:, kcp, 0:G_GP], op=OP.mult)
                        nc.vector.tensor_tensor(
                            out=p0[:, :, G_GP:ROWS], in0=p0[:, :, G_GP:ROWS],
                            in1=keep_sb[:, kcp, G_GP:ROWS], op=OP.mult)
                do_av(KC // 2 - 1, prev_pair)

                # pair boundary: kproj of the next pair rides freed slots
                if mc + 1 < MC:
                    kbufs[mc + 1] = kp.tile([P, S], bf, tag="kbuf",
                                            name=f"kbuf_{mc + 1}")
                    kproj_group(mc + 1, 0, tag="sB")  # sB frees first
                rs_t = small.tile([DK + 1, 2 * ROWS], bf, tag="rs", bufs=1,
                                  name=f"rs_{mc}")
                for hh in range(2):
                    for qh in range(2):
                        nc.scalar.activation(
                            out=rs_t[DK:DK + 1,
                                     hh * ROWS + qh * NS:hh * ROWS + (qh + 1) * NS],
                            in_=ctx[hh][qh][DK:DK + 1, :], func=AF.Copy)
                rsT = small.tile([P, 2 * ROWS // P], bf, tag="rsT",
                                 name=f"rsT_{mc}")
                rsT_tiles[mc] = rsT
                nc.sync.dma_start(out=rsT, in_=rs_t[DK:DK + 1, :])
                for g, (hh, qh) in enumerate(((0, 0), (0, 1), (1, 0), (1, 1))):
                    pr = slice(hh * DK, (hh + 1) * DK)
                    with nc.allow_low_precision(reason="fp8 ctxT"):
                        nc.scalar.activation(
                            out=ctxT[pr, mc, qh * NS:(qh + 1) * NS],
                            in_=ctx[hh][qh][0:DK, :], func=AF.Copy)
                    if mc + 1 < MC and g >= 1:
                        kproj_group(mc + 1, g)
            # preload sqrt table set while the last bounce is in flight
            sq_warm = small.tile([1, 1], f32, tag="sqw")
            nc.scalar.activation(out=sq_warm, in_=eps_t[0:1, 0:1], func=AF.Sqrt,
                                 bias=eps_t[0:1, 0:1], scale=1.0)
            recip_pair(MC - 1)

            # ---- phase 3: out projection (fp8 DR) + residual + LayerNorm
            # DR pairs (0,1),(2,3),(4,5): the last pair's ctxT chunk (5, whose
            # normalization lands latest) stays in the final group; rc0/rc1's
            # first two groups run under the last normalize chain.
            op_ps = {}

            def oproj_part(rc, gs, last=False):
                if rc not in op_ps:
                    op_ps[rc] = psp.tile([P, D], f32,
                                         tag="sA" if rc % 2 == 0 else "sB",
                                         name=f"op_{rc}")
                for g in gs:
                    for n0 in (0, NS):
                        n1 = min(n0 + NS, D)
                        nc.tensor.matmul(
                            op_ps[rc][:, n0:n1],
                            ctxT[:, 2 * g:2 * g + 2, rc * P:(rc + 1) * P],
                            w_sb["wo"][:, 2 * g:2 * g + 2, n0:n1],
                            start=(g == 0), stop=False, perf_mode=DR)
                if last:
                    # residual add (x256 host-side) via identity matmul
                    for n0 in (0, NS):
                        n1 = min(n0 + NS, D)
                        nc.tensor.matmul(op_ps[rc][:, n0:n1], ident_sb,
                                         qres_sb[:, rc, n0:n1],
                                         start=False, stop=True)

            oproj_part(0, range(2))
            oproj_part(1, range(2))
            finish_pair(MC - 1)
            nsub = 2
            sub = D // nsub  # 384 <= BN_STATS_FMAX
            for rc in range(RC):
                if rc < 2:
                    oproj_part(rc, [2], last=True)
                else:
                    oproj_part(rc, range(3), last=True)
                ps = op_ps.pop(rc)

                x_t = ph3.tile([P, D], bf, tag="x")
                nc.scalar.activation(out=x_t, in_=ps, func=AF.Copy)
                stats = small.tile([P, nsub, 6], f32, tag="stats")
                for sg in range(nsub):
                    nc.vector.bn_stats(out=stats[:, sg, :],
                                       in_=x_t[:, sg * sub:(sg + 1) * sub])
                mv = small.tile([P, 2], f32, tag="mv")
                nc.vector.bn_aggr(out=mv, in_=stats)
                std_t = small.tile([P, 1], f32, tag="std")
                nc.scalar.activation(out=std_t, in_=mv[:, 1:2], func=AF.Sqrt,
                                     bias=eps_t, scale=1.0)
                nc.vector.reciprocal(out=std_t, in_=std_t)
                xn = ph3.tile([P, D], bf, tag="xn")
                nc.vector.tensor_scalar(out=xn, in0=x_t, scalar1=mv[:, 0:1],
                                        scalar2=std_t, op0=OP.subtract,
                                        op1=OP.mult)
                xg = ph3.tile([P, D], bf, tag="xg")
                nc.gpsimd.tensor_tensor(out=xg, in0=xn, in1=gb, op=OP.mult)
                xf = ph3.tile([P, D], f32, tag="xf")
                nc.vector.tensor_tensor(out=xf, in0=xg, in1=bb, op=OP.add)
                nc.sync.dma_start(out=out_d.ap()[rc * P:(rc + 1) * P, :], in_=xf)

    nc.compile()
    return nc


def _get_nc():
    if "nc" not in _cached:
        _cached["nc"] = _build()
    return _cached["nc"]


def _make_in_maps(Q, Kt, V, attn_mask, Wq, bq, Wk, bk, Wv, bv, Wo, bo, gamma, beta):
    f32 = np.float32
    FP8 = ml_dtypes.float8_e4m3
    # fp8 DoubleRow projections: weights x8 / activations /8 so products
    # stay in range; wo x8 with the ctx x32 scale cancelled by LayerNorm.
    w = {"wq": (np.ascontiguousarray(Wq, f32) * 8).astype(FP8),
         "wk": (np.ascontiguousarray(Wk, f32) * 8).astype(FP8),
         "wv": (np.ascontiguousarray(Wv, f32) * 8).astype(FP8),
         "wo": (np.ascontiguousarray(Wo, f32) * 8).astype(FP8)}
    b = {"bq": np.ascontiguousarray(bq, f32) * np.float32(LOG2E),
         "bk": np.ascontiguousarray(bk, f32),
         "gamma": np.ascontiguousarray(gamma, f32),
         "beta": np.ascontiguousarray(beta, f32)}
    # fold bv and bo into the residual: out = attnV@Wo + (Q + bo + bv@Wo)
    res_bias = (np.asarray(bo, f32) + np.asarray(bv, f32) @ np.asarray(Wo, f32))
    in_maps = []
    for c in range(NCORES):
        bidx, half = divmod(c, 2)
        rows = slice(half * ROWS, (half + 1) * ROWS)
        m = {
            "qt": (np.ascontiguousarray(Q[bidx, rows].T) / 8).astype(FP8),
            "kt": (np.ascontiguousarray(Kt[bidx].T) / 8).astype(FP8),
            "vt": (np.ascontiguousarray(V[bidx].T) / 8).astype(FP8),
            "keep": np.ascontiguousarray(
                (~attn_mask[bidx, rows]).T.astype(BF16)),
            "qres": ((np.ascontiguousarray(Q[bidx, rows], f32) + res_bias)
                     * np.float32(QRES_SCALE)).astype(BF16),
        }
        m["ident"] = np.eye(P, dtype=BF16)
        m.update(w)
        m.update(b)
        in_maps.append(m)
    return in_maps


def kernel(Q, K, V, attn_mask, Wq, bq, Wk, bk, Wv, bv, Wo, bo, gamma, beta,
           _profile=None):
    from concourse.bass_utils import run_bass_kernel_spmd

    nc = _get_nc()
    in_maps = _make_in_maps(np.asarray(Q, np.float32), np.asarray(K, np.float32),
                            np.asarray(V, np.float32), np.asarray(attn_mask),
                            Wq, bq, Wk, bk, Wv, bv, Wo, bo, gamma, beta)
    kwargs = dict(_profile) if _profile else {}
    res = run_bass_kernel_spmd(nc, in_maps, list(range(NCORES)), **kwargs)
    if _profile is not None:
        _cached["last_results"] = res
    out = np.empty((B, S, D), np.float32)
    for c, m in enumerate(res.results):
        bidx, half = divmod(c, 2)
        out[bidx, half * ROWS:(half + 1) * ROWS] = m["out"]
    return out


# revision 15
# speedup vs baseline: 1.3460x; 1.0952x over previous
"""Trainium2 Bass kernel for nn_MultiHeadAttention (B=4, S=2048, D=768, H=12).

Sharding: query-parallel. 8 cores = 4 batches x 2 query-halves. Each core
computes full K/V projections for its batch plus Q projection / attention /
output projection / LayerNorm for its 1024 query rows. No collectives.

v2 design (vs the ACT-bound v1): the softmax pointwise work is split across
engines so no single engine is saturated:
  head0 of each pair: ACT exp -> fp8e4 p, mask-mult split GP/DVE by columns.
  head1 of each pair: ONE DVE scalar_tensor_tensor: i8 = RNE((log2e*s + 56)
    * keep), bitcast to fp8e4 => p ~= 2^((i8-56)/8) = e^(s/sqrt(dk)).
    (masked -> i8 0 -> +0.0; int8 convert is RNE+saturating, HW-verified)
Both heads' p are fp8 => AV matmuls run fp8 DoubleRow over key-chunk pairs
(half the PE issue time). Rowsum via a 65th ones-column (1/32) in the fp8
V-hat weights; reciprocal gives 32/rs so ctxT is x32-scaled into fp8 range.
Output projection is fp8 DoubleRow (Wo x8 host-side, residual x256); the
resulting 256x psum scale cancels in LayerNorm (scale-invariant; EPS x65536).
All 16 vproj chunks moved to phase 1 (attention has no PE slack anymore).
"""

import sys

for _p in ("/opt/trn_rl_repo", "/root/.axon_site/_ro/trn_rl_repo"):
    if _p not in sys.path:
        sys.path.insert(0, _p)

import numpy as np
import ml_dtypes

B = 4
S = 2048
D = 768
H = 12
DK = 64
NCORES = 8
ROWS = S // 2          # 1024 query rows per core
P = 128
KO = D // P            # 6 contraction chunks
MC = D // P            # 6 head-pair chunks
KC = S // P            # 16 key chunks
RC = ROWS // P         # 8 row chunks
VW = 68                # 64 v cols + ones col + 3 pad (H*VW % 16 == 0 for DR)
EPS = 1e-5
NS = 512               # PSUM bank = 512 f32; matmul out must stay in one bank

LOG2E = 1.4426950408889634
SC_ACT = 1.0 / (8.0 * LOG2E)   # undo log2e scale, apply 1/sqrt(dk)
B_I8 = 56.0                    # fp8e4m3 exponent bias offset (2^0 at i8=56)
ONESC = 1.0 / 32.0             # ones column: rowsum/32 -> recip = 32/rs
QRES_SCALE = 256.0             # 32 (ctx) * 8 (wo) psum scale
EPS_S = EPS * QRES_SCALE * QRES_SCALE
G_GP = 320                     # head0 mask cols on GPSIMD; rest on DVE

BF16 = ml_dtypes.bfloat16

_cached = {}


def _build():
    import concourse.bass as bass
    import concourse.tile as tile
    import concourse.mybir as mybir
    from concourse import bacc

    f32 = mybir.dt.float32
    bf = mybir.dt.bfloat16
    f8 = mybir.dt.float8e4
    i8 = mybir.dt.int8
    AF = mybir.ActivationFunctionType
    OP = mybir.AluOpType
    DR = mybir.MatmulPerfMode.DoubleRow

    nc = bacc.Bacc("TRN2", target_bir_lowering=False, debug=False)

    qt_d = nc.dram_tensor("qt", [D, ROWS], f8, kind="ExternalInput")
    kt_d = nc.dram_tensor("kt", [D, S], f8, kind="ExternalInput")
    vt_d = nc.dram_tensor("vt", [D, S], f8, kind="ExternalInput")
    keep_d = nc.dram_tensor("keep", [S, ROWS], bf, kind="ExternalInput")
    qres_d = nc.dram_tensor("qres", [ROWS, D], bf, kind="ExternalInput")
    w_d = {n: nc.dram_tensor(n, [D, D], f8, kind="ExternalInput")
           for n in ("wq", "wk", "wv", "wo")}
    ident_d = nc.dram_tensor("ident", [P, P], bf, kind="ExternalInput")
    b_d = {n: nc.dram_tensor(n, [D], f32, kind="ExternalInput")
           for n in ("bq", "bk", "gamma", "beta")}
    out_d = nc.dram_tensor("out", [ROWS, D], f32, kind="ExternalOutput")

    rs2_d = [nc.dram_tensor(f"rs2_bounce{mc}", [2, ROWS], bf, kind="Internal")
             for mc in range(MC)]

    def bcast_ap(handle, n, row=0):
        ap = handle.ap()
        return bass.AP(tensor=ap.tensor, offset=row * n, ap=[[0, DK], [1, n]])

    def bcast_ap_p(handle, n):
        ap = handle.ap()
        return bass.AP(tensor=ap.tensor, offset=0, ap=[[0, P], [1, n]])

    with tile.TileContext(nc) as tc:
        with tc.tile_pool(name="wp", bufs=1) as wp, \
             tc.tile_pool(name="xin", bufs=2) as xin, \
             tc.tile_pool(name="kp", bufs=2) as kp, \
             tc.tile_pool(name="ktp", bufs=2) as ktp, \
             tc.tile_pool(name="big", bufs=1) as big, \
             tc.tile_pool(name="pp", bufs=2) as ppool, \
             tc.tile_pool(name="small", bufs=2) as small, \
             tc.tile_pool(name="ph3", bufs=2) as ph3, \
             tc.tile_pool(name="ps", bufs=1, space="PSUM") as psp:

            def s_tile(tag, name):
                return psp.tile([P, ROWS], f32, tag=tag, name=name)

            # ---- PE warm-up in the DMA shadow (~4us to reach HAM 8/8) ----
            warm = wp.tile([P, P], bf, tag="warm")
            nc.vector.memset(warm, 0.0)
            wps = psp.tile([P, NS], f32, tag="sA", name="warm_ps")
            for i in range(36):
                nc.tensor.matmul(wps[:, 0:P], warm, warm,
                                 start=(i == 0), stop=(i == 35))

            # ---- weights / inputs, in PE consumption order ----
            w_sb = {}

            def load_w(n):
                t = wp.tile([P, KO, D], f8, tag=n, name=f"w_{n}")
                src = w_d[n].ap().rearrange("(o p) n -> p o n", p=P)
                for ko in range(KO):
                    nc.sync.dma_start(out=t[:, ko, :], in_=src[:, ko, :])
                w_sb[n] = t

            bq_sb = wp.tile([P, MC], f32, tag="bq")
            bk_sb = wp.tile([P, MC], f32, tag="bk")
            nc.sync.dma_start(out=bq_sb, in_=b_d["bq"].ap().rearrange("(o p) -> p o", p=P))
            nc.sync.dma_start(out=bk_sb, in_=b_d["bk"].ap().rearrange("(o p) -> p o", p=P))
            eps_t = wp.tile([P, 1], f32, tag="eps")
            nc.vector.memset(eps_t, EPS_S)

            # phase 1a: wq/qt interleaved on sync so the PE starts ASAP;
            # wk/kt on the scalar queue, wv/vt on the gpsimd queue, keep +
            # phase-3 tensors on the vector queue — four queues in parallel.
            wq_t = wp.tile([P, KO, D], f8, tag="wq", name="w_wq")
            w_sb["wq"] = wq_t
            wq_src = w_d["wq"].ap().rearrange("(o p) n -> p o n", p=P)
            qt_sb = xin.tile([P, KO, ROWS], f8, tag="xin", name="qt_sb")
            qt_src = qt_d.ap().rearrange("(o p) r -> p o r", p=P)
            for ko in range(KO):
                nc.sync.dma_start(out=wq_t[:, ko, :], in_=wq_src[:, ko, :])
                nc.sync.dma_start(out=qt_sb[:, ko, :], in_=qt_src[:, ko, :])

            wk_t = wp.tile([P, KO, D], f8, tag="wk", name="w_wk")
            w_sb["wk"] = wk_t
            wk_src = w_d["wk"].ap().rearrange("(o p) n -> p o n", p=P)
            for ko in range(KO):
                nc.scalar.dma_start(out=wk_t[:, ko, :], in_=wk_src[:, ko, :])
            kt_sb = []
            kt_src = kt_d.ap().rearrange("(o p) s -> p o s", p=P)
            for half in range(2):
                t = ktp.tile([P, KO, ROWS], f8, tag="kt", name=f"kt_sb{half}")
                for ko in range(KO):
                    nc.scalar.dma_start(
                        out=t[:, ko, :],
                        in_=kt_src[:, ko, half * ROWS:(half + 1) * ROWS])
                kt_sb.append(t)

            wv_t = wp.tile([P, KO, D], f8, tag="wv", name="w_wv")
            w_sb["wv"] = wv_t
            wv_src = w_d["wv"].ap().rearrange("(o p) n -> p o n", p=P)
            for ko in range(KO):
                nc.gpsimd.dma_start(out=wv_t[:, ko, :], in_=wv_src[:, ko, :])
            vt_sb = []
            vt_src = vt_d.ap().rearrange("(o p) s -> p o s", p=P)
            for half in range(2):
                t = xin.tile([P, KO, ROWS], f8, tag="xin", name=f"vt_sb{half}")
                for ko in range(KO):
                    nc.gpsimd.dma_start(
                        out=t[:, ko, :],
                        in_=vt_src[:, ko, half * ROWS:(half + 1) * ROWS])
                vt_sb.append(t)

            # keep mask: chunks 0-3 on the scalar queue (behind kt), 4-15 on
            # the gpsimd queue (behind vt) — paced ahead of attention use
            keep_sb = big.tile([P, KC, ROWS], bf, tag="keep")
            keep_src = keep_d.ap().rearrange("(c p) r -> p c r", p=P)
            for c in range(4):
                nc.scalar.dma_start(out=keep_sb[:, c, :], in_=keep_src[:, c, :])
            for c in range(4, KC):
                nc.gpsimd.dma_start(out=keep_sb[:, c, :], in_=keep_src[:, c, :])

            load_w("wo")
            gb = wp.tile([P, D], bf, tag="gb")
            bb = wp.tile([P, D], bf, tag="bb")
            nc.gpsimd.dma_start(out=gb, in_=bcast_ap_p(b_d["gamma"], D))
            nc.gpsimd.dma_start(out=bb, in_=bcast_ap_p(b_d["beta"], D))
            ident_sb = wp.tile([P, P], bf, tag="ident")
            nc.sync.dma_start(out=ident_sb, in_=ident_d.ap())
            qres_sb = big.tile([P, RC, D], bf, tag="qres")
            qres_src = qres_d.ap().rearrange("(c p) d -> p c d", p=P)
            for rc in range(RC):
                nc.sync.dma_start(out=qres_sb[:, rc, :], in_=qres_src[:, rc, :])

            # ---- phase 1: qhat = log2e * (q + bq), bf16 ----
            qhat = big.tile([P, MC, ROWS], bf, tag="qhat")
            for mc in range(MC):
                ps = s_tile("sA" if mc % 2 == 0 else "sB", f"qh_{mc}")
                for kp2 in range(KO // 2):
                    for n0 in (0, NS):
                        nc.tensor.matmul(
                            ps[:, n0:n0 + NS],
                            w_sb["wq"][:, 2 * kp2:2 * kp2 + 2, mc * P:(mc + 1) * P],
                            qt_sb[:, 2 * kp2:2 * kp2 + 2, n0:n0 + NS],
                            start=(kp2 == 0), stop=(kp2 == KO // 2 - 1),
                            perf_mode=DR)
                nc.scalar.activation(out=qhat[:, mc, :], in_=ps, func=AF.Identity,
                                     bias=bq_sb[:, mc:mc + 1], scale=LOG2E)

            # ---- kproj: kbuf[mc] = k~^T + bk (bf16), copies on ACT ----
            kbufs = [None] * MC

            def kproj_group(mc, g, tag="cx"):
                half, q0 = divmod(g, 2)
                ps = psp.tile([P, NS], f32, tag=tag, bufs=4 if tag == "cx" else 1,
                              name=f"kp_{mc}_{g}")
                for kp2 in range(KO // 2):
                    nc.tensor.matmul(
                        ps,
                        w_sb["wk"][:, 2 * kp2:2 * kp2 + 2, mc * P:(mc + 1) * P],
                        kt_sb[half][:, 2 * kp2:2 * kp2 + 2, q0 * NS:(q0 + 1) * NS],
                        start=(kp2 == 0), stop=(kp2 == KO // 2 - 1),
                        perf_mode=DR)
                nc.scalar.activation(out=kbufs[mc][:, g * NS:(g + 1) * NS],
                                     in_=ps, func=AF.Identity,
                                     bias=bk_sb[:, mc:mc + 1], scale=1.0)

            kbufs[0] = kp.tile([P, S], bf, tag="kbuf", name="kbuf_0")
            for g in range(4):
                kproj_group(0, g)

            # ---- v~ fp8 with ones column (all 16 chunks in phase 1) ----
            vhat = big.tile([P, KC, H * VW], f8, tag="vhat")
            vh4 = vhat.rearrange("p c (h w) -> p c h w", w=VW)
            with nc.allow_low_precision(reason="fp8 vhat"):
                nc.vector.memset(vh4[:, :, :, DK:DK + 1], ONESC)

            def vproj(kc, tag):
                half, c = divmod(kc, KC // 2)
                ps = psp.tile([P, D], f32, tag=tag, name=f"vp_{kc}")
                for kp2 in range(KO // 2):
                    for n0 in (0, NS):
                        n1 = min(n0 + NS, D)
                        nc.tensor.matmul(
                            ps[:, n0:n1],
                            vt_sb[half][:, 2 * kp2:2 * kp2 + 2, c * P:(c + 1) * P],
                            w_sb["wv"][:, 2 * kp2:2 * kp2 + 2, n0:n1],
                            start=(kp2 == 0), stop=(kp2 == KO // 2 - 1),
                            perf_mode=DR)
                with nc.allow_low_precision(reason="fp8 vhat"):
                    nc.scalar.activation(
                        out=vh4[:, kc, :, 0:DK],
                        in_=ps.rearrange("p (h w) -> p h w", w=DK),
                        func=AF.Copy)

            for kc in range(KC):
                vproj(kc, "sA" if kc % 2 == 0 else "sB")

            # ---- attention: head pairs ----------------------------------
            ctxT = big.tile([P, MC, ROWS], f8, tag="ctxT")
            rsb_tiles = {}
            rsT_tiles = {}

            def recip_pair(mc):
                rsT = rsT_tiles.pop(mc)
                with nc.allow_low_precision(reason="softmax rowsum recip bf16"):
                    nc.vector.reciprocal(out=rsT, in_=rsT)
                nc.sync.dma_start(
                    out=rs2_d[mc].ap().rearrange("a b -> (a b)")
                    .rearrange("(p o) -> p o", p=P), in_=rsT)
                rsb = small.tile([P, ROWS], bf, tag="rsb", name=f"rsb_{mc}")
                rsb_tiles[mc] = rsb
                for hh in range(2):
                    nc.gpsimd.dma_start(
                        out=rsb[hh * DK:(hh + 1) * DK, :],
                        in_=bcast_ap(rs2_d[mc], ROWS, row=hh))

            def finish_pair(mc):
                # normalize ctxT for pair mc (x 32/rowsum)
                with nc.allow_low_precision(reason="fp8 ctxT"):
                    nc.vector.tensor_tensor(out=ctxT[:, mc, :], in0=ctxT[:, mc, :],
                                            in1=rsb_tiles.pop(mc), op=OP.mult)

            for mc in range(MC):
                kbuf = kbufs[mc]
                ctx = [[psp.tile([DK + 1, NS], f32, tag="cx", bufs=4,
                                 name=f"ctx_{mc}_{hh}_{qh}")
                        for qh in range(2)] for hh in range(2)]
                p_pair = [None, None]  # fp8 [P, 2, ROWS] per head, per kc-pair
                prev_pair = None

                def do_av(j, pair_tiles):
                    # head0: bf16 p, plain MMs; head1: fp8 p, DoubleRow
                    h0 = 2 * mc
                    for jj2 in range(2):
                        for qh in range(2):
                            nc.tensor.matmul(
                                ctx[0][qh],
                                vh4[:, 2 * j + jj2, h0, 0:DK + 1],
                                pair_tiles[0][:, jj2, qh * NS:(qh + 1) * NS],
                                start=(j == 0 and jj2 == 0),
                                stop=(j == KC // 2 - 1 and jj2 == 1))
                    for qh in range(2):
                        nc.tensor.matmul(
                            ctx[1][qh],
                            vh4[:, 2 * j:2 * j + 2, 2 * mc + 1, 0:DK + 1],
                            pair_tiles[1][:, :, qh * NS:(qh + 1) * NS],
                            start=(j == 0), stop=(j == KC // 2 - 1),
                            perf_mode=DR)

                for kc in range(KC):
                    # deferred rowsum recip / normalize, off critical path
                    if kc == 5 and mc > 0:
                        recip_pair(mc - 1)
                    if kc == 11 and mc > 0:
                        finish_pair(mc - 1)


                    # concurrent row-tiled scores for the two heads
                    s_ps = [s_tile("sA", f"s_{mc}_{kc}_0"),
                            s_tile("sB", f"s_{mc}_{kc}_1")]
                    for n0 in (0, NS):
                        for hh in range(2):
                            pr = slice(hh * DK, (hh + 1) * DK)
                            nc.tensor.matmul(s_ps[hh][:, n0:n0 + NS],
                                             kbuf[pr, kc * P:(kc + 1) * P],
                                             qhat[pr, mc, n0:n0 + NS],
                                             start=True, stop=True)
                    # AV for the previous completed kc-pair (lags pointwise)
                    if kc % 2 == 0:
                        if kc >= 2:
                            do_av(kc // 2 - 1, prev_pair)
                        prev_pair = [
                            ppool.tile([P, 2, ROWS], bf, tag="p0",
                                       name=f"p_{mc}_{kc // 2}_0"),
                            ppool.tile([P, 2, ROWS], f8, tag="p1",
                                       name=f"p_{mc}_{kc // 2}_1")]
                    jj = kc & 1
                    p0, p1 = prev_pair

                    # head0: ACT exp -> bf16 (mask applied per kc-pair below)
                    nc.scalar.activation(out=p0[:, jj, :], in_=s_ps[0],
                                         func=AF.Exp, scale=SC_ACT)
                    # head1: one-op exp2 bit-trick with fused mask
                    with nc.allow_low_precision(reason="fp8 softmax"):
                        nc.vector.scalar_tensor_tensor(
                            out=p1[:, jj, :].bitcast(i8), in0=s_ps[1],
                            scalar=B_I8, in1=keep_sb[:, kc, :],
                            op0=OP.add, op1=OP.mult)
                    if jj == 1:
                        # head0 mask, batched over the kc-pair (bf16 2x DVE)
                        kcp = slice(kc - 1, kc + 1)
                        nc.vector.tensor_tensor(
                            out=p0[:, :, :], in0=p0[:, :, :],
                            in1=keep_sb[:, kcp, :], op=OP.mult)
                do_av(KC // 2 - 1, prev_pair)

                # pair boundary: kproj group 0 of the next pair rides the
                # freed sB slot (groups 1-3 are staggered into the pair);
                # ctx copies first so the cx slots free early, rs-row after
                # (its consumer, recip, is deferred to kc5 anyway).
                if mc + 1 < MC:
                    kbufs[mc + 1] = kp.tile([P, S], bf, tag="kbuf",
                                            name=f"kbuf_{mc + 1}")
                    kproj_group(mc + 1, 0, tag="sB")  # sB frees first
                rs_t = small.tile([DK + 1, 2 * ROWS], bf, tag="rs", bufs=1,
                                  name=f"rs_{mc}")
                for g, (hh, qh) in enumerate(((0, 0), (0, 1), (1, 0), (1, 1))):
                    pr = slice(hh * DK, (hh + 1) * DK)
                    with nc.allow_low_precision(reason="fp8 ctxT"):
                        nc.scalar.activation(
                            out=ctxT[pr, mc, qh * NS:(qh + 1) * NS],
                            in_=ctx[hh][qh][0:DK, :], func=AF.Copy)
                    nc.scalar.activation(
                        out=rs_t[DK:DK + 1,
                                 hh * ROWS + qh * NS:hh * ROWS + (qh + 1) * NS],
                        in_=ctx[hh][qh][DK:DK + 1, :], func=AF.Copy)
                    if mc + 1 < MC and g >= 1:
                        kproj_group(mc + 1, g)
                rsT = small.tile([P, 2 * ROWS // P], bf, tag="rsT",
                                 name=f"rsT_{mc}")
                rsT_tiles[mc] = rsT
                nc.sync.dma_start(out=rsT, in_=rs_t[DK:DK + 1, :])
            # preload sqrt table set while the last bounce is in flight
            sq_warm = small.tile([1, 1], f32, tag="sqw")
            nc.scalar.activation(out=sq_warm, in_=eps_t[0:1, 0:1], func=AF.Sqrt,
                                 bias=eps_t[0:1, 0:1], scale=1.0)
            recip_pair(MC - 1)

            # ---- phase 3: out projection (fp8 DR) + residual + LayerNorm
            # DR pairs (0,1),(2,3),(4,5): the last pair's ctxT chunk (5, whose
            # normalization lands latest) stays in the final group; rc0/rc1's
            # first two groups run under the last normalize chain.
            op_ps = {}

            def oproj_part(rc, gs, last=False):
                if rc not in op_ps:
                    op_ps[rc] = psp.tile([P, D], f32,
                                         tag="sA" if rc % 2 == 0 else "sB",
                                         name=f"op_{rc}")
                for g in gs:
                    for n0 in (0, NS):
                        n1 = min(n0 + NS, D)
                        nc.tensor.matmul(
                            op_ps[rc][:, n0:n1],
                            ctxT[:, 2 * g:2 * g + 2, rc * P:(rc + 1) * P],
                            w_sb["wo"][:, 2 * g:2 * g + 2, n0:n1],
                            start=(g == 0), stop=False, perf_mode=DR)
                if last:
                    # residual add (x256 host-side) via identity matmul
                    for n0 in (0, NS):
                        n1 = min(n0 + NS, D)
                        nc.tensor.matmul(op_ps[rc][:, n0:n1], ident_sb,
                                         qres_sb[:, rc, n0:n1],
                                         start=False, stop=True)

            oproj_part(0, range(2))
            oproj_part(1, range(2))
            finish_pair(MC - 1)
            nsub = 2
            sub = D // nsub  # 384 <= BN_STATS_FMAX
            for rc in range(RC):
                if rc < 2:
                    oproj_part(rc, [2], last=True)
                else:
                    oproj_part(rc, range(3), last=True)
                ps = op_ps.pop(rc)

                x_t = ph3.tile([P, D], bf, tag="x")
                nc.scalar.activation(out=x_t, in_=ps, func=AF.Copy)
                stats = small.tile([P, nsub, 6], f32, tag="stats")
                for sg in range(nsub):
                    nc.vector.bn_stats(out=stats[:, sg, :],
                                       in_=x_t[:, sg * sub:(sg + 1) * sub])
                mv = small.tile([P, 2], f32, tag="mv")
                nc.vector.bn_aggr(out=mv, in_=stats)
                std_t = small.tile([P, 1], f32, tag="std")
                nc.scalar.activation(out=std_t, in_=mv[:, 1:2], func=AF.Sqrt,
                                     bias=eps_t, scale=1.0)
                nc.vector.reciprocal(out=std_t, in_=std_t)
                xn = ph3.tile([P, D], bf, tag="xn")
                nc.vector.tensor_scalar(out=xn, in0=x_t, scalar1=mv[:, 0:1],
                                        scalar2=std_t, op0=OP.subtract,
                                        op1=OP.mult)
                xg = ph3.tile([P, D], bf, tag="xg")
                nc.vector.tensor_tensor(out=xg, in0=xn, in1=gb, op=OP.mult)
                xf = ph3.tile([P, D], f32, tag="xf")
                nc.vector.tensor_tensor(out=xf, in0=xg, in1=bb, op=OP.add)
                nc.sync.dma_start(out=out_d.ap()[rc * P:(rc + 1) * P, :], in_=xf)

    nc.compile()
    return nc


def _get_nc():
    if "nc" not in _cached:
        _cached["nc"] = _build()
    return _cached["nc"]


def _make_in_maps(Q, Kt, V, attn_mask, Wq, bq, Wk, bk, Wv, bv, Wo, bo, gamma, beta):
    f32 = np.float32
    FP8 = ml_dtypes.float8_e4m3
    # fp8 DoubleRow projections: weights x8 / activations /8 so products
    # stay in range; wo x8 with the ctx x32 scale cancelled by LayerNorm.
    w = {"wq": (np.ascontiguousarray(Wq, f32) * 8).astype(FP8),
         "wk": (np.ascontiguousarray(Wk, f32) * 8).astype(FP8),
         "wv": (np.ascontiguousarray(Wv, f32) * 8).astype(FP8),
         "wo": (np.ascontiguousarray(Wo, f32) * 8).astype(FP8)}
    b = {"bq": np.ascontiguousarray(bq, f32) * np.float32(LOG2E),
         "bk": np.ascontiguousarray(bk, f32),
         "gamma": np.ascontiguousarray(gamma, f32),
         "beta": np.ascontiguousarray(beta, f32)}
    # fold bv and bo into the residual: out = attnV@Wo + (Q + bo + bv@Wo)
    res_bias = (np.asarray(bo, f32) + np.asarray(bv, f32) @ np.asarray(Wo, f32))
    in_maps = []
    for c in range(NCORES):
        bidx, half = divmod(c, 2)
        rows = slice(half * ROWS, (half + 1) * ROWS)
        m = {
            "qt": (np.ascontiguousarray(Q[bidx, rows].T) / 8).astype(FP8),
            "kt": (np.ascontiguousarray(Kt[bidx].T) / 8).astype(FP8),
            "vt": (np.ascontiguousarray(V[bidx].T) / 8).astype(FP8),
            "keep": np.ascontiguousarray(
                (~attn_mask[bidx, rows]).T.astype(BF16)),
            "qres": ((np.ascontiguousarray(Q[bidx, rows], f32) + res_bias)
                     * np.float32(QRES_SCALE)).astype(BF16),
        }
        m["ident"] = np.eye(P, dtype=BF16)
        m.update(w)
        m.update(b)
        in_maps.append(m)
    return in_maps


def kernel(Q, K, V, attn_mask, Wq, bq, Wk, bk, Wv, bv, Wo, bo, gamma, beta,
           _profile=None):
    from concourse.bass_utils import run_bass_kernel_spmd

    nc = _get_nc()
    in_maps = _make_in_maps(np.asarray(Q, np.float32), np.asarray(K, np.float32),
                            np.asarray(V, np.float32), np.asarray(attn_mask),
                            Wq, bq, Wk, bk, Wv, bv, Wo, bo, gamma, beta)
    kwargs = dict(_profile) if _profile else {}
    res = run_bass_kernel_spmd(nc, in_maps, list(range(NCORES)), **kwargs)
    if _profile is not None:
        _cached["last_results"] = res
    out = np.empty((B, S, D), np.float32)
    for c, m in enumerate(res.results):
        bidx, half = divmod(c, 2)
        out[bidx, half * ROWS:(half + 1) * ROWS] = m["out"]
    return out
